# revision 1
# baseline (speedup 1.0000x reference)
"""Trainium2 Bass kernel for a 2-layer relational GraphSAGE VGAE encoder.

Contract: kernel(**inputs) takes the FULL unsharded inputs (as produced by
setup_inputs()) and returns the full (mu, logvar) tuple.

Strategy (8 NeuronCores, SPMD single NEFF):
  - Nodes block-sharded: core c owns nodes [c*2500, (c+1)*2500), padded to 2560.
  - Edges partitioned by destination-node owner, chunked into 128-edge chunks
    per (relation, node-group) cell; chunk counts are shared across cores
    (max) so one NEFF serves all cores SPMD.
  - Layer-1 segment-mean: the gathered x[src] rows are a pure function of the
    host inputs, so the host materializes them per chunk slot (xga) and the
    kernel streams them with plain dense DMA (no on-device gather). Each
    chunk costs 4 matmuls vs a host-built one-hot rhs [128 edges, 512 dst]
    carrying 1/cnt values, emitting the feature-major mean directly in PSUM.
  - Dense layer-1 feature-major [hid, nodes] fp16 with fp32 PSUM; the x-side
    (Wr@x) matmuls run first so the mean PSUM->SBUF copies hide under them.
    PSUM-region-major matmul ordering throughout. BatchNorm (eval) is folded
    into the layer-2 weights on the host.
  - Layer-2 projections are computed NODE-major (out[node, ch]) by using the
    feature-major relu tiles as lhsT, so the AllGather input needs no PE
    transposes; biases are added via host-broadcast [128,512] tiles.
  - The projected tab features are AllGather'd per node group (fp16); the
    last AllGather is emitted before the self-side projections, which hide it.
  - Layer-2 aggregation is node-major: per 128-dst block, one batched
    dma_gather (SWDGE, mlp library, int16 indices) pulls the projected rows
    of the edge sources from the all-gathered table; the host-built one-hot
    [128 edges, 128 dst] (1/cnt folded in) is the stationary lhsT so each
    chunk costs ONE 512-wide matmul. Blocks are interleaved into the
    self-projection groups so PE consumes them as gathers land. Output is
    node-major [2560, 512] fp32, reassembled on the host.
"""
import sys

sys.path.insert(0, "/opt/trn_rl_repo")

import numpy as np

NCORES = 8
N = 20000
E = 100000
IN = 512
HID = 512
CAT = 2560
OUT = 256
BN_EPS = 1e-5

NLOC = N // NCORES          # 2500
NPAD = 2560                 # 20 * 128, 5 * 512
NG = NPAD // 512            # 5 node groups of 512 per core
NB = NPAD // 128            # 20 dst blocks of 128 per core
NREL = 5
P = 128
TABROWS = NCORES * NPAD     # 20480


# ----------------------------------------------------------------------------
# Host-side preprocessing: sharding, edge chunking, weight folding
# ----------------------------------------------------------------------------

def _chunk_edges(key, ncells, rowidx, col, inv, width):
    """Group edges by per-core cell, chunk each cell into 128-edge chunks.

    key: [E] int = core * ncells + cell   (cell < ncells)
    rowidx: [E] gather row index for each edge (must fit int16)
    col: [E] one-hot column in [0, width) (dst position within cell span)
    inv: [E] f32 one-hot value (1/cnt)

    Returns: nch [ncells] shared chunk counts (max over cores, >=1),
             base [ncells] chunk base offsets, Ctot,
             idx16 [NCORES, 128, Ctot*8] int16 (dma_gather index layout:
               slot i=(chunk*128+p) at [i%16, i//16], tiled to 128 rows),
             vals [NCORES, 128, Ctot*width] f16 one-hot values (pad rows 0)
    """
    counts = np.bincount(key, minlength=NCORES * ncells).reshape(NCORES, ncells)
    nch = np.maximum((counts + P - 1) // P, 1).max(axis=0)  # [ncells]
    base = np.concatenate([[0], np.cumsum(nch)[:-1]])
    Ctot = int(nch.sum())

    order = np.argsort(key, kind="stable")
    ks = key[order]
    first_of_run = np.r_[True, ks[1:] != ks[:-1]]
    run_starts = np.flatnonzero(first_of_run)
    run_id = np.cumsum(first_of_run) - 1
    pos = np.arange(len(ks)) - run_starts[run_id]

    core_s = ks // ncells
    cell_s = ks % ncells
    chunk_s = base[cell_s] + pos // P
    row_s = pos % P

    i_flat = chunk_s * P + row_s
    idx16 = np.zeros((NCORES, 16, Ctot * 8), np.int16)
    idx16[core_s, i_flat % 16, i_flat // 16] = rowidx[order].astype(np.int16)
    idx16 = np.ascontiguousarray(np.tile(idx16, (1, 8, 1)))

    # host-built one-hot values, partition-major: [core, 128, Ctot*width]
    vals = np.zeros((NCORES, P, Ctot, width), np.float16)
    vals[core_s, row_s, chunk_s, col[order]] = inv[order]
    vals = vals.reshape(NCORES, P, Ctot * width)
    return nch, base, Ctot, idx16, vals


def _preprocess(x, edge_index, edge_attr, Wl5, Wr5, bl5,
                Wmu_l, Wmu_r, bmu, Wlv_l, Wlv_r, blv,
                gamma, beta, run_mean, run_var):
    x = np.asarray(x, np.float32)
    src = np.asarray(edge_index[0], np.int64)
    dst = np.asarray(edge_index[1], np.int64)
    rel = np.asarray(edge_attr, np.int64)

    # --- per-node degree counts ---
    cnt1 = np.bincount(rel * N + dst, minlength=NREL * N).reshape(NREL, N)
    inv1 = 1.0 / np.maximum(cnt1, 1.0)
    cnt2 = np.bincount(dst, minlength=N)
    inv2 = 1.0 / np.maximum(cnt2, 1.0)

    core = dst // NLOC
    loc = dst % NLOC
    g = loc // 512

    # layer-1 cells: (rel, group); gather rows straight from x table;
    # one-hot col is position within the 512-wide group
    key1 = (core * NREL + rel) * NG + g
    nch1, base1, C1, a1i, v1 = _chunk_edges(
        key1, NREL * NG, src, loc % 512, inv1[rel, dst], 512)

    # layer-2 cells: (128-dst-block); gather rows from the all-gathered table.
    # The table is assembled by NG per-group AllGathers, so its row layout is
    # [g][core][col]: row = g*8*512 + core*512 + col.
    src_loc = src % NLOC
    tabrow = ((src_loc // 512) * (NCORES * 512) + (src // NLOC) * 512
              + src_loc % 512)
    key2 = core * NB + loc // 128
    nch2, base2, C2, a2i, v2 = _chunk_edges(
        key2, NB, tabrow, loc % 128, inv2[dst], 128)

    # --- node features ---
    xtab = x.astype(np.float16)                           # [N, 512] gather table
    xt = np.zeros((NCORES, IN, NPAD), np.float16)         # feature-major local x
    for c in range(NCORES):
        xt[c, :, :NLOC] = x[c * NLOC:(c + 1) * NLOC].T
    # partition-major: xtP[c][p, g*2048 + kc*512 + f] = xt[c][kc*128+p, g*512+f]
    xtP = np.ascontiguousarray(
        xt.reshape(NCORES, 4, P, NG, 512).transpose(0, 2, 3, 1, 4)
        .reshape(NCORES, P, NG * 2048))

    # --- weight folding (BN eval folded into layer-2 weights) ---
    f64 = np.float64
    s = np.asarray(gamma, f64) / np.sqrt(np.asarray(run_var, f64) + BN_EPS)
    t = np.asarray(beta, f64) - np.asarray(run_mean, f64) * s

    # partition-major weightT: w[k][p, kc*512 + j] = W^T[k][kc*128+p, j]
    def _pmaj_w(W5):
        wt = np.asarray(W5, np.float32).transpose(0, 2, 1).astype(np.float16)
        return np.ascontiguousarray(
            wt.reshape(NREL, 4, P, HID).transpose(0, 2, 1, 3)
            .reshape(NREL, P, 4 * HID))
    wlt = _pmaj_w(Wl5)
    wrt = _pmaj_w(Wr5)

    Wtab = np.concatenate([np.asarray(Wmu_l, f64), np.asarray(Wlv_l, f64)], 0)
    Wself = np.concatenate([np.asarray(Wmu_r, f64), np.asarray(Wlv_r, f64)], 0)
    Wall = np.concatenate([Wtab * s[None, :], Wself * s[None, :]], 0)  # [1024, 2560]
    # partition-major: wallt[p, r*1024 + j] = Wall.T[r*128+p, j]
    wallt = np.ascontiguousarray(
        Wall.T.astype(np.float16).reshape(20, P, 1024).transpose(1, 0, 2)
        .reshape(P, 20 * 1024))

    tW = (Wtab @ t).astype(np.float32)                                  # [512]
    bself = (Wself @ t + np.concatenate(
        [np.asarray(bmu, f64), np.asarray(blv, f64)])).astype(np.float32)

    # layer-1 bias tile [128, 20]: column k*4+mc is the per-partition bias
    blb = np.ascontiguousarray(
        np.asarray(bl5, np.float32).reshape(NREL * 4, P).T)   # [128, 20]
    # broadcast bias tiles for the node-major projections: every row = bias
    twbb = np.ascontiguousarray(np.broadcast_to(tW, (P, 512)))
    bsbb = np.ascontiguousarray(np.broadcast_to(bself, (P, 512)))

    # L1 gathered features are a pure function of host inputs: materialize
    # x[src] per chunk slot on the host and DMA it densely (no on-device
    # gather for layer 1). xga[c][p, ci*512+f] = x[idx(c, ci, p), f].
    a1i32 = np.zeros((NCORES, P, C1), np.int64)
    for c in range(NCORES):
        sl = a1i[c][:16, :].reshape(16, C1, 8)
        for i16r in range(16):
            for col8 in range(8):
                a1i32[c, col8 * 16 + i16r, :] = sl[i16r, :, col8]
    xga = xtab[a1i32].reshape(NCORES, P, C1 * 512)

    meta = (tuple(nch1), tuple(base1), C1, tuple(nch2), tuple(base2), C2)
    in_maps = []
    for c in range(NCORES):
        in_maps.append({
            "xga": xga[c], "xt": xtP[c], "v1": v1[c],
            "a2i": a2i[c], "v2": v2[c],
            "wlt": wlt, "wrt": wrt, "wallt": wallt,
            "blb": blb, "twbb": twbb, "bsbb": bsbb,
        })
    return meta, in_maps


# ----------------------------------------------------------------------------
# Device kernel
# ----------------------------------------------------------------------------

def _build(meta):
    import concourse.bacc as bacc
    import concourse.bass as bass
    import concourse.tile as tile
    import concourse.mybir as mybir
    from concourse import library_config

    nch1, base1, C1, nch2, base2, C2 = meta
    nch1 = np.asarray(nch1).reshape(NREL, NG)
    base1 = np.asarray(base1).reshape(NREL, NG)
    nch2 = np.asarray(nch2)
    base2 = np.asarray(base2)
    A1MAX = int(nch1.max())
    A2MAX = int(nch2.max())

    f16, f32, i16, i32 = (mybir.dt.float16, mybir.dt.float32,
                          mybir.dt.int16, mybir.dt.int32)
    ADD, MAX = mybir.AluOpType.add, mybir.AluOpType.max

    nc = bacc.Bacc("TRN2", target_bir_lowering=False, debug=False,
                   num_devices=NCORES, num_swdge_queues=4)

    xga_t = nc.dram_tensor("xga", [P, C1 * 512], f16, kind="ExternalInput")
    xt_t = nc.dram_tensor("xt", [P, NG * 2048], f16, kind="ExternalInput")
    v1_t = nc.dram_tensor("v1", [P, C1 * 512], f16, kind="ExternalInput")
    a2i_t = nc.dram_tensor("a2i", [P, C2 * 8], i16, kind="ExternalInput")
    v2_t = nc.dram_tensor("v2", [P, C2 * 128], f16, kind="ExternalInput")
    wlt_t = nc.dram_tensor("wlt", [NREL, P, 4 * HID], f16, kind="ExternalInput")
    wrt_t = nc.dram_tensor("wrt", [NREL, P, 4 * HID], f16, kind="ExternalInput")
    wallt_t = nc.dram_tensor("wallt", [P, 20 * 1024], f16, kind="ExternalInput")
    blb_t = nc.dram_tensor("blb", [P, NREL * 4], f32, kind="ExternalInput")
    twbb_t = nc.dram_tensor("twbb", [P, 512], f32, kind="ExternalInput")
    bsbb_t = nc.dram_tensor("bsbb", [P, 512], f32, kind="ExternalInput")
    out_t = nc.dram_tensor("out", [NPAD, 512], f32, kind="ExternalOutput")

    hrelu = nc.dram_tensor("hrelu", [P, NG * 20 * 512], f16, kind="Internal")
    warm_t = nc.dram_tensor("warm", [P, 512], f16, kind="Internal")
    ag_in = nc.dram_tensor("ag_in", [NPAD, 512], f16, kind="Internal")
    ag_tab = nc.dram_tensor("ag_tab", [TABROWS, 512], f16,
                            kind="Internal", addr_space="Shared")

    with tile.TileContext(nc) as tc:
        with (
            tc.tile_pool(name="constp", bufs=1) as constp,
            tc.tile_pool(name="resp", bufs=1) as resp,
            tc.tile_pool(name="wp", bufs=2) as wp,
            tc.tile_pool(name="iop", bufs=3) as iop,
            tc.tile_pool(name="ohp", bufs=4) as ohp,
            tc.tile_pool(name="actp", bufs=2) as actp,
            tc.tile_pool(name="psum", bufs=2, space="PSUM") as pp,
        ):
            # ---- constants / resident tiles ----
            blb_sb = constp.tile([P, NREL * 4], f32, name="blb_sb", tag="blb")
            nc.sync.dma_start(out=blb_sb[:], in_=blb_t.ap())
            twbb_sb = constp.tile([P, 512], f32, name="twbb_sb", tag="twbb")
            nc.sync.dma_start(out=twbb_sb[:], in_=twbb_t.ap())
            bsbb_sb = constp.tile([P, 512], f32, name="bsbb_sb", tag="bsbb")
            nc.sync.dma_start(out=bsbb_sb[:], in_=bsbb_t.ap())

            # layer-2 gather indices + the Q7 ucode library (needed by
            # dma_gather) load up front on the otherwise-idle gpsimd queue
            nc.gpsimd.load_library(library_config.mlp)
            a2i_sb = resp.tile([P, C2 * 8], i16, name="a2i_sb", tag="a2i")
            nc.gpsimd.dma_start(out=a2i_sb[:], in_=a2i_t.ap())

            # local x (p-major) + stacked projection weightT, resident.
            xt_sb = resp.tile([P, NG * 2048], f16, name="xt_sb", tag="xt")
            wall_sb = resp.tile([P, 20 * 1024], f16, name="wall_sb", tag="wall")

            # ---- PE warm-up: a few throwaway matmuls while DMAs fill ----
            wu = constp.tile([P, 512], f16, name="wu", tag="wu")
            nc.vector.memset(wu[:], 0.0)
            wu_ps = pp.tile([P, 2048], f32, space="PSUM", name="wu_ps", tag="big")
            for i in range(36):
                nc.tensor.matmul(out=wu_ps[:, 0:512], lhsT=wu[:, 0:P],
                                 rhs=wu[:], start=(i == 0), stop=(i == 35))
            nc.vector.tensor_copy(out=wu[:], in_=wu_ps[:, 0:512])
            nc.sync.dma_start(out=warm_t.ap(), in_=wu[:])

            def load_vals(vt, width, cbase, nchunks, nametag, bufs=3):
                av = ohp.tile([P, width * (A1MAX if width == 512 else A2MAX)],
                              f16, name=nametag, tag=f"oh{width}", bufs=bufs)
                nc.scalar.dma_start(
                    out=av[:, :nchunks * width],
                    in_=vt.ap()[:, cbase * width:(cbase + nchunks) * width])
                return av

            # ====== Phase 1+2a fused, g-outer: SAGE layer 1 -> node-major
            # projections -> per-group AllGather (collectives overlap P1).
            def emit_ag(g):
                nc.gpsimd.collective_compute(
                    "AllGather", mybir.AluOpType.bypass,
                    replica_groups=[list(range(NCORES))],
                    ins=[ag_in.ap()[g * 512:(g + 1) * 512, :]],
                    outs=[ag_tab.ap()[g * NCORES * 512:
                                      (g + 1) * NCORES * 512, :]])

            pending_ag = None
            for gg in range(NG):
                if gg > 0:
                    nc.scalar.dma_start(
                        out=xt_sb[:, gg * 2048:(gg + 1) * 2048],
                        in_=xt_t.ap()[:, gg * 2048:(gg + 1) * 2048])
                rts = []
                for k in range(NREL):
                    if gg == 0 and k == 4:
                        nc.scalar.dma_start(out=wall_sb[:], in_=wallt_t.ap())
                    wl = wp.tile([P, 4 * 512], f16, name=f"wl_{gg}_{k}", tag="wl")
                    nc.sync.dma_start(out=wl[:], in_=wlt_t.ap()[k])
                    wr = wp.tile([P, 4 * 512], f16, name=f"wr_{gg}_{k}", tag="wr")
                    nc.sync.dma_start(out=wr[:], in_=wrt_t.ap()[k])

                    nchunks = int(nch1[k, gg])
                    cbase = int(base1[k, gg])
                    # --- aggregation: mean_k^T for this node group ---
                    gth = iop.tile([P, A1MAX * 512], f16, name=f"g1_{k}_{gg}",
                                   tag="gth", bufs=2)
                    nc.scalar.dma_start(
                        out=gth[:, :nchunks * 512],
                        in_=xga_t.ap()[:, cbase * 512:(cbase + nchunks) * 512])
                    if gg == 0 and k == 0:
                        nc.scalar.dma_start(
                            out=xt_sb[:, 0:2048], in_=xt_t.ap()[:, 0:2048])
                    av = load_vals(v1_t, 512, cbase, nchunks, f"av1_{k}_{gg}")
                    mean_ps = pp.tile([P, 2048], f32, space="PSUM",
                                      name=f"agg_{k}_{gg}", tag="big")
                    for cc in range(4):
                        for ci in range(nchunks):
                            nc.tensor.matmul(
                                out=mean_ps[:, cc * 512:(cc + 1) * 512],
                                lhsT=gth[:, ci * 512 + cc * P:
                                         ci * 512 + (cc + 1) * P],
                                rhs=av[:, ci * 512:(ci + 1) * 512],
                                start=(ci == 0), stop=(ci == nchunks - 1))
                    mean_sb = []
                    for cc in range(4):
                        m = actp.tile([P, 512], f16, name=f"mean_{k}_{gg}_{cc}",
                                      tag=f"mean{cc}")
                        nc.vector.tensor_copy(
                            out=m[:], in_=mean_ps[:, cc * 512:(cc + 1) * 512])
                        mean_sb.append(m)

                    # --- dense: h = relu(Wl@mean + Wr@x + b) ---
                    h_ps = pp.tile([P, 2048], f32, space="PSUM",
                                   name=f"h_{k}_{gg}", tag="big")
                    for mc in range(4):
                        for kc in range(4):
                            nc.tensor.matmul(
                                out=h_ps[:, mc * 512:(mc + 1) * 512],
                                lhsT=wr[:, kc * 512 + mc * P:kc * 512 + (mc + 1) * P],
                                rhs=xt_sb[:, gg * 2048 + kc * 512:
                                          gg * 2048 + (kc + 1) * 512],
                                start=(kc == 0), stop=False)
                        for kc in range(4):
                            nc.tensor.matmul(
                                out=h_ps[:, mc * 512:(mc + 1) * 512],
                                lhsT=wl[:, kc * 512 + mc * P:kc * 512 + (mc + 1) * P],
                                rhs=mean_sb[kc][:], start=False, stop=(kc == 3))
                    rt = actp.tile([P, 2048], f16, name=f"relu_{gg}_{k}",
                                   tag=f"rt{k}", bufs=1)
                    for mc in range(4):
                        nc.vector.tensor_scalar(
                            out=rt[:, mc * 512:(mc + 1) * 512],
                            in0=h_ps[:, mc * 512:(mc + 1) * 512],
                            scalar1=blb_sb[:, k * 4 + mc:k * 4 + mc + 1],
                            scalar2=0.0,
                            op0=ADD, op1=MAX)
                    nc.scalar.dma_start(
                        out=hrelu.ap()[:, (gg * 20 + k * 4) * 512:
                                       (gg * 20 + k * 4 + 4) * 512],
                        in_=rt[:])
                    rts.append(rt)

                if pending_ag is not None:
                    emit_ag(pending_ag)
                    pending_ag = None

                # --- aggregated-side projections, node-major ---
                p_ps = pp.tile([P, 2048], f32, space="PSUM",
                               name=f"proj_{gg}", tag="big")
                for nb in range(4):
                    o = p_ps[:, nb * 512:(nb + 1) * 512]
                    for r in range(20):
                        nc.tensor.matmul(
                            out=o,
                            lhsT=rts[r // 4][:, (r % 4) * 512 + nb * P:
                                             (r % 4) * 512 + (nb + 1) * P],
                            rhs=wall_sb[:, r * 1024:r * 1024 + 512],
                            start=(r == 0), stop=(r == 19))
                    tab = actp.tile([P, 512], f16, name=f"tab_{gg}_{nb}",
                                    tag=f"tab{nb}", bufs=2)
                    nc.vector.tensor_tensor(
                        out=tab[:], in0=o, in1=twbb_sb[:], op=ADD)
                    nc.scalar.dma_start(
                        out=ag_in.ap()[gg * 512 + nb * P:
                                       gg * 512 + (nb + 1) * P, :],
                        in_=tab[:])
                pending_ag = gg

            # ====== Phase 3a + 3b interleaved: self-side projections
            # (node-major) overlap the tail AllGather; layer-2 aggregation
            # blocks are emitted between 3a groups once their gathers (which
            # wait on the last AllGather) can have landed.
            if pending_ag is not None:
                emit_ag(pending_ag)
                pending_ag = None
            sf_all = []
            m2_state = {"ps": None}

            def emit_l2_block(b):
                nchunks = int(nch2[b])
                cbase = int(base2[b])
                g2 = iop.tile([P, A2MAX * 512], f16, name=f"g2_{b}",
                              tag="g2", bufs=4)
                nc.gpsimd.dma_gather(
                    g2[:, :nchunks * 512].rearrange("p (c f) -> p c f", f=512),
                    ag_tab.ap(),
                    a2i_sb[:, cbase * 8:(cbase + nchunks) * 8],
                    nchunks * P, nchunks * P, 512,
                    queue_num=b % 4)
                av2 = load_vals(v2_t, 128, cbase, nchunks, f"av2_{b}", bufs=4)
                if b % 4 == 0:
                    m2_state["ps"] = pp.tile([P, 2048], f32, space="PSUM",
                                             name=f"m2_{b}", tag="big")
                o = m2_state["ps"][:, (b % 4) * 512:(b % 4 + 1) * 512]
                for ci in range(nchunks):
                    nc.tensor.matmul(
                        out=o, lhsT=av2[:, ci * 128:(ci + 1) * 128],
                        rhs=g2[:, ci * 512:(ci + 1) * 512],
                        start=(ci == 0), stop=(ci == nchunks - 1))
                ob = actp.tile([P, 512], f32, name=f"out_{b}", tag="outsb",
                               bufs=2)
                nc.vector.tensor_tensor(
                    out=ob[:], in0=o, in1=sf_all[b][:], op=ADD)
                nc.scalar.dma_start(
                    out=out_t.ap()[b * P:(b + 1) * P, :], in_=ob[:])

            for gg in range(NG):
                rts = []
                for k in range(NREL):
                    rt = actp.tile([P, 2048], f16, name=f"hrB_{gg}_{k}",
                                   tag=f"rt{k}", bufs=1)
                    nc.scalar.dma_start(
                        out=rt[:],
                        in_=hrelu.ap()[:, (gg * 20 + k * 4) * 512:
                                       (gg * 20 + k * 4 + 4) * 512])
                    rts.append(rt)
                p_ps = pp.tile([P, 2048], f32, space="PSUM",
                               name=f"self_{gg}", tag="big")
                for nb in range(4):
                    o = p_ps[:, nb * 512:(nb + 1) * 512]
                    for r in range(20):
                        nc.tensor.matmul(
                            out=o,
                            lhsT=rts[r // 4][:, (r % 4) * 512 + nb * P:
                                             (r % 4) * 512 + (nb + 1) * P],
                            rhs=wall_sb[:, r * 1024 + 512:(r + 1) * 1024],
                            start=(r == 0), stop=(r == 19))
                    sf = actp.tile([P, 512], f16, name=f"selfr_{gg}_{nb}",
                                   tag=f"sf{gg * 4 + nb}", bufs=1)
                    nc.vector.tensor_tensor(
                        out=sf[:], in0=o, in1=bsbb_sb[:], op=ADD)
                    sf_all.append(sf)
                if gg == 3:
                    for b in range(0, 4):
                        emit_l2_block(b)
                elif gg == 4:
                    for b in range(4, 8):
                        emit_l2_block(b)
            for b in range(8, NB):
                emit_l2_block(b)

    nc.compile()
    return nc


# ----------------------------------------------------------------------------
# Entry point
# ----------------------------------------------------------------------------

_CACHE = {}


def build_and_run(inputs, trace=False, trace_kwargs=None):
    from concourse import bass_utils

    meta, in_maps = _preprocess(**inputs)
    if meta not in _CACHE:
        _CACHE[meta] = _build(meta)
    nc = _CACHE[meta]
    res = bass_utils.run_bass_kernel_spmd(
        nc, in_maps, core_ids=list(range(NCORES)),
        trace=trace, **(trace_kwargs or {}))

    mu = np.empty((N, OUT), np.float32)
    lv = np.empty((N, OUT), np.float32)
    for c in range(NCORES):
        raw = res.results[c]["out"]            # [NPAD, 512] f32 node-major
        mu[c * NLOC:(c + 1) * NLOC] = raw[:NLOC, :OUT]
        lv[c * NLOC:(c + 1) * NLOC] = raw[:NLOC, OUT:]
    return (mu, lv), res


def kernel(**inputs):
    out, _ = build_and_run(inputs, trace=False)
    return out

